# revision 5
# baseline (speedup 1.0000x reference)
"""APPNP (GNN message passing) on 8 Trainium2 NeuronCores — v2.

vs v1: maxsub-degree-sorted tiles (1.39x slot padding vs 1.91x), tile-major
node positions, consolidated gather instructions (<=126 cols each, ~40/step
vs 903/step), persistent weight table, per-group idx DMA, group-wide
multiply + per-(tile,sub) f32 reduces.
"""

import inspect
import math
import re
import sys

import numpy as np

if "/opt/trn_rl_repo" not in sys.path:
    sys.path.insert(0, "/opt/trn_rl_repo")

from concourse import bass, bacc, mybir  # noqa: E402
from concourse.tile import TileContext  # noqa: E402
from concourse.bass_utils import run_bass_kernel_spmd  # noqa: E402

N = 100000
F_IN = 512
NHID = 64
NCLS = 40
ALPHA = 0.1
K_STEPS = 10
NCORES = 8

EW = 128          # table row width in bf16 (256B stride)
NQUEUES = 4
MAX_COLS = 8      # per-gather column cap (1024 idxs/instr hardware limit)
GROUP_SLOTS = 140  # target slot columns per tile-group (SBUF budget)


def _make_patched_gather():
    src = inspect.getsource(bass.BassGpSimd.dma_gather)
    src = inspect.cleandoc("def dma_gather" + src.split("def dma_gather", 1)[1])
    src = re.sub(
        r"assert \(\s*elem_size_bytes > 0 and elem_size_bytes % 256 == 0\s*\)",
        "assert elem_size_bytes > 0",
        src,
    )
    assert "% 256 == 0" not in src.split("stride_bytes_256")[0]
    ns = vars(bass).copy()
    exec(src, ns)
    return ns["dma_gather"]


_patched_dma_gather = _make_patched_gather()


class Plan:
    def __init__(self):
        self.n = N
        self.ncls = NCLS
        self.nhid = NHID
        self.f_in = F_IN
        self.rpc = N // NCORES           # 12500
        self.tiles = math.ceil(self.rpc / 128)   # 98
        self.rp = self.tiles * 128       # 12544
        self.ntab = NCORES * self.rp     # 100352
        self.nsub = self.ntab // 4       # 25088 (= 2*rp, core-pair subtables)
        assert self.nsub <= 32768


def preprocess(p, edge_index, edge_values):
    dest = np.asarray(edge_index[0], np.int64)
    src = np.asarray(edge_index[1], np.int64)
    w = np.asarray(edge_values, np.float32) * (1.0 - ALPHA)

    core = dest // p.rpc
    local = dest - core * p.rpc
    sub = src // (2 * p.rpc)                     # source's core-pair = subtable

    degs = np.zeros((NCORES, p.rpc, 4), np.int64)
    np.add.at(degs, (core, local, sub), 1)
    deg = degs.sum(axis=2)
    mx = degs.max(axis=1 + 1)                    # [NCORES, rpc] max over subs

    sortpos = np.empty((NCORES, p.rpc), np.int64)
    for c in range(NCORES):
        dv = degs[c]
        order = np.lexsort((dv[:, 3], dv[:, 2], dv[:, 1], dv[:, 0],
                            dv[:, 3] // 2, dv[:, 2] // 2, dv[:, 1] // 2,
                            dv[:, 0] // 2, -mx[c]))
        sortpos[c, order] = np.arange(p.rpc)

    # tile-major positions: pos = t*128 + part
    s_core = src // p.rpc
    pos_s = sortpos[s_core, src - s_core * p.rpc]
    idx16 = (s_core % 2) * p.rp + pos_s          # row within subtable

    pos_d = sortpos[core, local]
    dt = pos_d // 128
    dp = pos_d - dt * 128

    counts = np.zeros((NCORES, p.tiles, 4, 128), np.int64)
    np.add.at(counts, (core, dt, sub, dp), 1)
    L = np.maximum(counts.max(axis=(0, 3)), 1)   # [tiles, 4]

    # groups: consecutive tiles while total slots stay <= GROUP_SLOTS
    groups = []
    t0 = 0
    while t0 < p.tiles:
        t1 = t0 + 1
        while t1 < p.tiles and L[t0:t1 + 1].sum() <= GROUP_SLOTS:
            t1 += 1
        groups.append((t0, t1))
        t0 = t1

    # column layout: [g0: s0 blocks t0..t1 | s1 | s2 | s3][g1: ...]
    col_off = np.zeros((p.tiles, 4), np.int64)
    ginfo = []                                   # per group: (t0,t1,[ (s, c0, ncols) x4 ], gcol0, gcols)
    cur = 0
    for (t0, t1) in groups:
        gcol0 = cur
        sblocks = []
        for s in range(4):
            c0 = cur
            for t in range(t0, t1):
                col_off[t, s] = cur
                cur += int(L[t, s])
            sblocks.append((s, c0, cur - c0))
        ginfo.append((t0, t1, sblocks, gcol0, cur - gcol0))
    total_slots = cur

    # slot rank within (core, tile, sub, part); secondary sort by table row
    grp = (((core * p.tiles + dt) * 4 + sub) * 128 + dp)
    sort_idx = np.lexsort((idx16, grp))
    grp_sorted = grp[sort_idx]
    starts = np.r_[0, np.flatnonzero(np.diff(grp_sorted)) + 1]
    gs = np.zeros(len(grp_sorted), np.int64)
    gs[starts] = starts
    gs = np.maximum.accumulate(gs)
    rank = np.empty(len(grp_sorted), np.int64)
    rank[sort_idx] = np.arange(len(grp_sorted)) - gs

    IDX = np.zeros((NCORES, 128, total_slots), np.int64)
    WG = np.zeros((NCORES, 128, total_slots), np.float32)
    colv = col_off[dt, sub] + rank
    IDX[core, dp, colv] = idx16
    WG[core, dp, colv] = w

    # wrapped idx image: instructions are <=MAX_COLS-column chunks of each
    # (group, sub) block; wrap each chunk [16, 8*cols] and replicate x8
    wrap_cols = 8 * total_slots
    IDXW = np.zeros((NCORES, 128, wrap_cols), np.int16)
    for (t0, t1, sblocks, gcol0, gcols) in ginfo:
        for (s, c0, ncols_blk) in sblocks:
            for cb in range(c0, c0 + ncols_blk, MAX_COLS):
                ncols = min(MAX_COLS, c0 + ncols_blk - cb)
                nidx = 128 * ncols
                ncw = nidx // 16
                ii = np.arange(nidx)
                rr, cc = ii % 16, ii // 16
                woff = 8 * cb
                for c in range(NCORES):
                    flat = IDX[c, :, cb:cb + ncols].T.reshape(-1)
                    w16 = np.zeros((16, ncw), np.int16)
                    w16[rr, cc] = flat.astype(np.int16)
                    IDXW[c, :, woff:woff + ncw] = np.tile(w16, (8, 1))

    import ml_dtypes
    WG = WG.astype(ml_dtypes.bfloat16)
    return dict(sortpos=sortpos, total_slots=total_slots, L=L, ginfo=ginfo,
                IDXW=IDXW, WG=WG)


def build_kernel(p, meta):
    tiles = p.tiles
    total_slots = int(meta["total_slots"])
    L = meta["L"]
    ginfo = meta["ginfo"]
    kchunks = p.f_in // 128
    ncls = p.ncls
    max_gslots = max(g[4] for g in ginfo)
    max_gt = max(g[1] - g[0] for g in ginfo)

    nc = bacc.Bacc("TRN2", target_bir_lowering=False, debug=False,
                   num_devices=NCORES, num_swdge_queues=NQUEUES)

    f32 = mybir.dt.float32
    bf16 = mybir.dt.bfloat16
    i16 = mybir.dt.int16
    featT = nc.declare_dram_parameter("featT", [kchunks, 128, p.rp], f32, isOutput=False)
    W1p = nc.declare_dram_parameter("W1p", [128, kchunks * p.nhid], f32, isOutput=False)
    b1p = nc.declare_dram_parameter("b1p", [p.nhid, 1], f32, isOutput=False)
    W2p = nc.declare_dram_parameter("W2p", [p.nhid, ncls], f32, isOutput=False)
    b2p = nc.declare_dram_parameter("b2p", [128, ncls], f32, isOutput=False)
    idxp = nc.declare_dram_parameter("idxp", [128, 8 * total_slots], i16, isOutput=False)
    wp = nc.declare_dram_parameter("wp", [128, total_slots], bf16, isOutput=False)
    outp = nc.declare_dram_parameter("out", [p.rp, ncls], f32, isOutput=True)

    shard = nc.dram_tensor("shard", [p.rp, EW], bf16)
    xtable = nc.dram_tensor("xtable", [p.ntab, EW], bf16, addr_space="Shared")
    rg = [list(range(NCORES))]
    qn = [0]

    with TileContext(nc) as tc, \
         nc.allow_low_precision(reason="bf16 propagation validated on baseline (rel 2.6e-3)"):
        with tc.tile_pool(name="persist", bufs=1) as pp:
            x_sb = pp.tile([128, tiles * EW], bf16)
            hp = pp.tile([128, tiles * ncls], bf16)
            w_sb = pp.tile([128, total_slots], bf16)
            ixall = pp.tile([128, 8 * total_slots], i16)
            nc.vector.memset(x_sb[:, :], 0.0)
            nc.sync.dma_start(out=w_sb[:, :], in_=wp[:, :])
            nc.sync.dma_start(out=ixall[:, :], in_=idxp[:, :])

            # ---- fc phase ----
            with tc.tile_pool(name="psum1", bufs=4, space="PSUM") as ps1, \
                 tc.tile_pool(name="psum2", bufs=4, space="PSUM") as ps2, \
                 tc.tile_pool(name="fcw", bufs=1) as fcw, \
                 tc.tile_pool(name="ft", bufs=4) as ftp, \
                 tc.tile_pool(name="x1", bufs=4) as x1p:
                w1sb = fcw.tile([128, kchunks * p.nhid], f32)
                nc.sync.dma_start(out=w1sb[:, :], in_=W1p[:, :])
                w2sb = fcw.tile([p.nhid, ncls], f32)
                nc.sync.dma_start(out=w2sb[:, :], in_=W2p[:, :])
                b1sb = fcw.tile([p.nhid, 1], f32)
                nc.sync.dma_start(out=b1sb[:, :], in_=b1p[:, :])
                b2sb = fcw.tile([128, ncls], f32)
                nc.sync.dma_start(out=b2sb[:, :], in_=b2p[:, :])

                nbatch = min(14, tiles)
                bsz = math.ceil(tiles / nbatch)
                for b in range(nbatch):
                    t0 = b * bsz
                    t1 = min(tiles, t0 + bsz)
                    if t0 >= t1:
                        continue
                    nrows = (t1 - t0) * 128
                    fts = []
                    for k in range(kchunks):
                        ft = ftp.tile([128, bsz * 128], f32, tag="ft")
                        nc.sync.dma_start(out=ft[:, :nrows],
                                          in_=featT[k, :, t0 * 128:t1 * 128])
                        fts.append(ft)
                    for t in range(t0, t1):
                        ro = (t - t0) * 128
                        psum1 = ps1.tile([p.nhid, 128], f32, tag="p1")
                        for k in range(kchunks):
                            nc.tensor.matmul(
                                psum1[:, :],
                                lhsT=w1sb[:, k * p.nhid:(k + 1) * p.nhid],
                                rhs=fts[k][:, ro:ro + 128],
                                start=(k == 0), stop=(k == kchunks - 1))
                        x1t = x1p.tile([p.nhid, 128], f32, tag="x1")
                        nc.scalar.activation(x1t[:, :], psum1[:, :],
                                             mybir.ActivationFunctionType.Relu,
                                             bias=b1sb[:, :1])
                        psum2 = ps2.tile([128, ncls], f32, tag="p2")
                        nc.tensor.matmul(psum2[:, :], lhsT=x1t[:, :], rhs=w2sb[:, :],
                                         start=True, stop=True)
                        nc.vector.tensor_tensor(
                            out=x_sb[:, t * EW:t * EW + ncls],
                            in0=psum2[:, :], in1=b2sb[:, :], op=mybir.AluOpType.add)
                        nc.vector.tensor_scalar_mul(
                            hp[:, t * ncls:(t + 1) * ncls],
                            x_sb[:, t * EW:t * EW + ncls], ALPHA)

            # ---- propagation ----
            def share_x():
                nc.sync.dma_start(
                    out=shard[:, :].rearrange('(t q) e -> q t e', q=128),
                    in_=x_sb[:, :].rearrange('p (t e) -> p t e', e=EW))
                nc.gpsimd.collective_compute(
                    "AllGather", mybir.AluOpType.bypass, replica_groups=rg,
                    ins=[shard[:, :]], outs=[xtable[:, :]])

            share_x()

            with tc.tile_pool(name="gout", bufs=6) as gp, \
                 tc.tile_pool(name="acc", bufs=2) as accp, \
                 tc.tile_pool(name="xadd", bufs=2) as xap:

                def do_group(gi):
                    t0, t1, sblocks, gcol0, gcols = gi
                    ngt = t1 - t0
                    g = gp.tile([128, max_gslots * ncls], bf16, tag="g")
                    for (s, c0, ncols_blk) in sblocks:
                        for cb in range(c0, c0 + ncols_blk, MAX_COLS):
                            ncols = min(MAX_COLS, c0 + ncols_blk - cb)
                            nidx = 128 * ncols
                            lo = (cb - gcol0) * ncls
                            _patched_dma_gather(
                                nc.gpsimd,
                                out_ap=g[:, lo:lo + ncols * ncls].rearrange(
                                    'p (s e) -> p s e', e=ncls),
                                in_ap=xtable[s * p.nsub:(s + 1) * p.nsub, :ncls],
                                idxs_ap=ixall[:, 8 * cb:8 * cb + nidx // 16],
                                num_idxs=nidx, num_idxs_reg=nidx,
                                elem_size=ncls, elem_step=EW,
                                queue_num=qn[0] % NQUEUES)
                            qn[0] += 1
                        # weighted multiply per sub-block (finer overlap)
                        lo = (c0 - gcol0) * ncls
                        nc.vector.tensor_tensor(
                            out=g[:, lo:lo + ncols_blk * ncls],
                            in0=g[:, lo:lo + ncols_blk * ncls],
                            in1=w_sb[:, c0:c0 + ncols_blk].to_broadcast(
                                [128, ncols_blk, ncls]),
                            op=mybir.AluOpType.mult)
                    # per-(tile, sub) reduce into acc[128, j, s, ncls] f32
                    acc = accp.tile([128, max_gt * 4 * ncls], f32, tag="acc")
                    for (s, c0, ncols) in sblocks:
                        cc = c0 - gcol0
                        for t in range(t0, t1):
                            l = int(L[t, s])
                            j = t - t0
                            nc.vector.tensor_reduce(
                                out=acc[:, (j * 4 + s) * ncls:(j * 4 + s + 1) * ncls],
                                in_=g[:, cc * ncls:(cc + l) * ncls].rearrange(
                                    'p (s e) -> p s e', e=ncls).transpose([0, 2, 1]),
                                axis=mybir.AxisListType.X, op=mybir.AluOpType.add)
                            cc += l
                    # combine 4 subs
                    xadd = xap.tile([128, max_gt * ncls], f32, tag="xa")
                    nc.vector.tensor_reduce(
                        out=xadd[:, :ngt * ncls],
                        in_=acc[:, :ngt * 4 * ncls].rearrange(
                            'p (j s e) -> p j s e', s=4, e=ncls).transpose([0, 1, 3, 2]),
                        axis=mybir.AxisListType.X, op=mybir.AluOpType.add)
                    # + alpha*h -> x_sb
                    nc.vector.tensor_tensor(
                        out=x_sb[:, :].rearrange('p (t e) -> p t e', e=EW)[:, t0:t1, :ncls],
                        in0=xadd[:, :ngt * ncls].rearrange('p (t e) -> p t e', e=ncls),
                        in1=hp[:, t0 * ncls:t1 * ncls].rearrange('p (t e) -> p t e', e=ncls),
                        op=mybir.AluOpType.add)

                for k in range(K_STEPS):
                    for gi in ginfo:
                        do_group(gi)
                    if k != K_STEPS - 1:
                        share_x()

            # ---- log_softmax ----
            with tc.tile_pool(name="smx", bufs=1) as smxp, \
                 tc.tile_pool(name="aggf", bufs=4) as aggp:
                xv16 = x_sb[:, :].rearrange('p (t e) -> p t e', e=EW)[:, :, :ncls]
                xf = smxp.tile([128, tiles * ncls], f32, tag="xf")
                nc.vector.tensor_copy(
                    out=xf[:, :].rearrange('p (t e) -> p t e', e=ncls), in_=xv16)
                xv = xf[:, :].rearrange('p (t e) -> p t e', e=ncls)
                sm = smxp.tile([128, tiles * ncls], f32, tag="sm")
                rmax = aggp.tile([128, tiles], f32, tag="aggf")
                nc.vector.tensor_reduce(out=rmax[:, :], in_=xv,
                                        axis=mybir.AxisListType.X, op=mybir.AluOpType.max)
                smv = sm[:, :tiles * ncls].rearrange('p (t e) -> p t e', e=ncls)
                nc.vector.tensor_tensor(
                    out=smv, in0=xv,
                    in1=rmax[:, :].to_broadcast([128, tiles, ncls]),
                    op=mybir.AluOpType.subtract)
                ex = smxp.tile([128, tiles * ncls], f32, tag="ex")
                nc.scalar.activation(ex[:, :], sm[:, :],
                                     mybir.ActivationFunctionType.Exp)
                ssum = aggp.tile([128, tiles], f32, tag="aggf")
                nc.vector.tensor_reduce(
                    out=ssum[:, :],
                    in_=ex[:, :].rearrange('p (t e) -> p t e', e=ncls),
                    axis=mybir.AxisListType.X, op=mybir.AluOpType.add)
                lsum = aggp.tile([128, tiles], f32, tag="aggf")
                nc.scalar.activation(lsum[:, :], ssum[:, :],
                                     mybir.ActivationFunctionType.Ln)
                nc.vector.tensor_tensor(
                    out=smv, in0=smv,
                    in1=lsum[:, :].to_broadcast([128, tiles, ncls]),
                    op=mybir.AluOpType.subtract)
                nc.sync.dma_start(
                    out=outp[:, :].rearrange('(t q) e -> q t e', q=128),
                    in_=sm[:, :].rearrange('p (t e) -> p t e', e=ncls))

    nc.compile()
    return nc


def run(features, edge_index, edge_values, W1, b1, W2, b2, trace=False):
    p = Plan()
    meta = preprocess(p, edge_index, edge_values)
    nc = build_kernel(p, meta)

    features = np.asarray(features, np.float32)
    W1 = np.asarray(W1, np.float32)
    b1 = np.asarray(b1, np.float32).reshape(-1)
    W2 = np.asarray(W2, np.float32)
    b2 = np.asarray(b2, np.float32).reshape(-1)
    kchunks = p.f_in // 128
    sortpos = meta["sortpos"]

    in_maps = []
    for c in range(NCORES):
        fpad = np.zeros((p.rp, p.f_in), np.float32)
        fpad[sortpos[c]] = features[c * p.rpc:(c + 1) * p.rpc]
        ft = np.ascontiguousarray(fpad.T).reshape(kchunks, 128, p.rp)
        in_maps.append({
            "featT": np.ascontiguousarray(ft),
            "W1p": np.ascontiguousarray(W1.reshape(kchunks, 128, p.nhid)
                                        .transpose(1, 0, 2).reshape(128, kchunks * p.nhid)),
            "b1p": np.ascontiguousarray(b1.reshape(p.nhid, 1)),
            "W2p": np.ascontiguousarray(W2),
            "b2p": np.ascontiguousarray(np.broadcast_to(b2, (128, p.ncls))),
            "idxp": np.ascontiguousarray(meta["IDXW"][c]),
            "wp": np.ascontiguousarray(meta["WG"][c]),
        })

    res = run_bass_kernel_spmd(nc, in_maps, list(range(NCORES)), trace=trace)

    out = np.empty((p.n, p.ncls), np.float32)
    for c in range(NCORES):
        o = res.results[c]["out"]
        out[c * p.rpc:(c + 1) * p.rpc] = o[sortpos[c]]
    return out, res


def kernel(**inputs):
    out, _ = run(inputs["features"], inputs["edge_index"], inputs["edge_values"],
                 inputs["W1"], inputs["b1"], inputs["W2"], inputs["b2"])
    return out


# revision 6
# speedup vs baseline: 1.0929x; 1.0929x over previous
"""APPNP (GNN message passing) on 8 Trainium2 NeuronCores — v2.

vs v1 (37.6ms -> ~17.9ms): degree-sorted tiles cut slot padding 1.91x->1.34x
(537k vs 763k gather idxs/step; desc-gen on GpSimd at ~3.4ns/idx is the wall),
tile-major positions, 543 full 8-column gather instrs/step (1024-idx HW cap),
idx table + weights persistent in SBUF (no per-step reloads), small tile-groups
(~140 slots) with 6-deep gather buffering to keep the gen stream fed, per-sub
weighted multiply + per-(tile,sub) f32 reduces + per-group sub-combine.
"""

import inspect
import math
import re
import sys

import numpy as np

if "/opt/trn_rl_repo" not in sys.path:
    sys.path.insert(0, "/opt/trn_rl_repo")

from concourse import bass, bacc, mybir  # noqa: E402
from concourse.tile import TileContext  # noqa: E402
from concourse.bass_utils import run_bass_kernel_spmd  # noqa: E402

N = 100000
F_IN = 512
NHID = 64
NCLS = 40
ALPHA = 0.1
K_STEPS = 10
NCORES = 8

EW = 128          # table row width in bf16 (256B stride)
NQUEUES = 4
MAX_COLS = 8      # per-gather column cap (1024 idxs/instr hardware limit)
GROUP_SLOTS = 140  # target slot columns per tile-group (SBUF budget)


def _make_patched_gather():
    src = inspect.getsource(bass.BassGpSimd.dma_gather)
    src = inspect.cleandoc("def dma_gather" + src.split("def dma_gather", 1)[1])
    src = re.sub(
        r"assert \(\s*elem_size_bytes > 0 and elem_size_bytes % 256 == 0\s*\)",
        "assert elem_size_bytes > 0",
        src,
    )
    assert "% 256 == 0" not in src.split("stride_bytes_256")[0]
    ns = vars(bass).copy()
    exec(src, ns)
    return ns["dma_gather"]


_patched_dma_gather = _make_patched_gather()


class Plan:
    def __init__(self):
        self.n = N
        self.ncls = NCLS
        self.nhid = NHID
        self.f_in = F_IN
        self.rpc = N // NCORES           # 12500
        self.tiles = math.ceil(self.rpc / 128)   # 98
        self.rp = self.tiles * 128       # 12544
        self.ntab = NCORES * self.rp     # 100352
        self.nsub = self.ntab // 4       # 25088 (= 2*rp, core-pair subtables)
        assert self.nsub <= 32768


def preprocess(p, edge_index, edge_values):
    dest = np.asarray(edge_index[0], np.int64)
    src = np.asarray(edge_index[1], np.int64)
    w = np.asarray(edge_values, np.float32) * (1.0 - ALPHA)

    core = dest // p.rpc
    local = dest - core * p.rpc
    sub = src // (2 * p.rpc)                     # source's core-pair = subtable

    degs = np.zeros((NCORES, p.rpc, 4), np.int64)
    np.add.at(degs, (core, local, sub), 1)
    deg = degs.sum(axis=2)
    mx = degs.max(axis=1 + 1)                    # [NCORES, rpc] max over subs

    sortpos = np.empty((NCORES, p.rpc), np.int64)
    for c in range(NCORES):
        dv = degs[c]
        order = np.lexsort((dv[:, 3], dv[:, 2], dv[:, 1], dv[:, 0],
                            dv[:, 3] // 2, dv[:, 2] // 2, dv[:, 1] // 2,
                            dv[:, 0] // 2, -mx[c]))
        sortpos[c, order] = np.arange(p.rpc)

    # tile-major positions: pos = t*128 + part
    s_core = src // p.rpc
    pos_s = sortpos[s_core, src - s_core * p.rpc]
    idx16 = (s_core % 2) * p.rp + pos_s          # row within subtable

    pos_d = sortpos[core, local]
    dt = pos_d // 128
    dp = pos_d - dt * 128

    counts = np.zeros((NCORES, p.tiles, 4, 128), np.int64)
    np.add.at(counts, (core, dt, sub, dp), 1)
    L = np.maximum(counts.max(axis=(0, 3)), 1)   # [tiles, 4]

    # groups: consecutive tiles while total slots stay <= GROUP_SLOTS
    groups = []
    t0 = 0
    while t0 < p.tiles:
        t1 = t0 + 1
        while t1 < p.tiles and L[t0:t1 + 1].sum() <= GROUP_SLOTS:
            t1 += 1
        groups.append((t0, t1))
        t0 = t1

    # column layout: [g0: s0 blocks t0..t1 | s1 | s2 | s3][g1: ...]
    col_off = np.zeros((p.tiles, 4), np.int64)
    ginfo = []                                   # per group: (t0,t1,[ (s, c0, ncols) x4 ], gcol0, gcols)
    cur = 0
    for (t0, t1) in groups:
        gcol0 = cur
        sblocks = []
        for s in range(4):
            c0 = cur
            for t in range(t0, t1):
                col_off[t, s] = cur
                cur += int(L[t, s])
            sblocks.append((s, c0, cur - c0))
        ginfo.append((t0, t1, sblocks, gcol0, cur - gcol0))
    total_slots = cur

    # slot rank within (core, tile, sub, part); secondary sort by table row
    grp = (((core * p.tiles + dt) * 4 + sub) * 128 + dp)
    sort_idx = np.lexsort((idx16, grp))
    grp_sorted = grp[sort_idx]
    starts = np.r_[0, np.flatnonzero(np.diff(grp_sorted)) + 1]
    gs = np.zeros(len(grp_sorted), np.int64)
    gs[starts] = starts
    gs = np.maximum.accumulate(gs)
    rank = np.empty(len(grp_sorted), np.int64)
    rank[sort_idx] = np.arange(len(grp_sorted)) - gs

    IDX = np.zeros((NCORES, 128, total_slots), np.int64)
    WG = np.zeros((NCORES, 128, total_slots), np.float32)
    colv = col_off[dt, sub] + rank
    IDX[core, dp, colv] = idx16
    WG[core, dp, colv] = w

    # wrapped idx image: instructions are <=MAX_COLS-column chunks of each
    # (group, sub) block; wrap each chunk [16, 8*cols] and replicate x8
    wrap_cols = 8 * total_slots
    IDXW = np.zeros((NCORES, 128, wrap_cols), np.int16)
    for (t0, t1, sblocks, gcol0, gcols) in ginfo:
        for (s, c0, ncols_blk) in sblocks:
            for cb in range(c0, c0 + ncols_blk, MAX_COLS):
                ncols = min(MAX_COLS, c0 + ncols_blk - cb)
                nidx = 128 * ncols
                ncw = nidx // 16
                ii = np.arange(nidx)
                rr, cc = ii % 16, ii // 16
                woff = 8 * cb
                for c in range(NCORES):
                    flat = IDX[c, :, cb:cb + ncols].T.reshape(-1)
                    w16 = np.zeros((16, ncw), np.int16)
                    w16[rr, cc] = flat.astype(np.int16)
                    IDXW[c, :, woff:woff + ncw] = np.tile(w16, (8, 1))

    import ml_dtypes
    WG = WG.astype(ml_dtypes.bfloat16)
    return dict(sortpos=sortpos, total_slots=total_slots, L=L, ginfo=ginfo,
                IDXW=IDXW, WG=WG)


def build_kernel(p, meta):
    tiles = p.tiles
    total_slots = int(meta["total_slots"])
    L = meta["L"]
    ginfo = meta["ginfo"]
    kchunks = p.f_in // 128
    ncls = p.ncls
    max_gslots = max(g[4] for g in ginfo)
    max_gt = max(g[1] - g[0] for g in ginfo)

    nc = bacc.Bacc("TRN2", target_bir_lowering=False, debug=False,
                   num_devices=NCORES, num_swdge_queues=NQUEUES)

    f32 = mybir.dt.float32
    bf16 = mybir.dt.bfloat16
    i16 = mybir.dt.int16
    featT = nc.declare_dram_parameter("featT", [kchunks, 128, p.rp], f32, isOutput=False)
    W1p = nc.declare_dram_parameter("W1p", [128, kchunks * p.nhid], f32, isOutput=False)
    b1p = nc.declare_dram_parameter("b1p", [p.nhid, 1], f32, isOutput=False)
    W2p = nc.declare_dram_parameter("W2p", [p.nhid, ncls], f32, isOutput=False)
    b2p = nc.declare_dram_parameter("b2p", [128, ncls], f32, isOutput=False)
    idxp = nc.declare_dram_parameter("idxp", [128, 8 * total_slots], i16, isOutput=False)
    wp = nc.declare_dram_parameter("wp", [128, total_slots], bf16, isOutput=False)
    outp = nc.declare_dram_parameter("out", [p.rp, ncls], f32, isOutput=True)

    shard = nc.dram_tensor("shard", [p.rp, EW], bf16)
    xtable = nc.dram_tensor("xtable", [p.ntab, EW], bf16, addr_space="Shared")
    rg = [list(range(NCORES))]
    qn = [0]

    with TileContext(nc) as tc, \
         nc.allow_low_precision(reason="bf16 propagation validated on baseline (rel 2.6e-3)"):
        with tc.tile_pool(name="persist", bufs=1) as pp:
            x_sb = pp.tile([128, tiles * EW], bf16)
            hp = pp.tile([128, tiles * ncls], bf16)
            w_sb = pp.tile([128, total_slots], bf16)
            ixall = pp.tile([128, 8 * total_slots], i16)
            nc.vector.memset(x_sb[:, :], 0.0)
            nc.sync.dma_start(out=w_sb[:, :], in_=wp[:, :])
            nc.sync.dma_start(out=ixall[:, :], in_=idxp[:, :])

            # ---- fc phase ----
            with tc.tile_pool(name="psum1", bufs=4, space="PSUM") as ps1, \
                 tc.tile_pool(name="psum2", bufs=4, space="PSUM") as ps2, \
                 tc.tile_pool(name="fcw", bufs=1) as fcw, \
                 tc.tile_pool(name="ft", bufs=4) as ftp, \
                 tc.tile_pool(name="x1", bufs=4) as x1p:
                w1sb = fcw.tile([128, kchunks * p.nhid], f32)
                nc.sync.dma_start(out=w1sb[:, :], in_=W1p[:, :])
                w2sb = fcw.tile([p.nhid, ncls], f32)
                nc.sync.dma_start(out=w2sb[:, :], in_=W2p[:, :])
                b1sb = fcw.tile([p.nhid, 1], f32)
                nc.sync.dma_start(out=b1sb[:, :], in_=b1p[:, :])
                b2sb = fcw.tile([128, ncls], f32)
                nc.sync.dma_start(out=b2sb[:, :], in_=b2p[:, :])

                nbatch = min(14, tiles)
                bsz = math.ceil(tiles / nbatch)
                for b in range(nbatch):
                    t0 = b * bsz
                    t1 = min(tiles, t0 + bsz)
                    if t0 >= t1:
                        continue
                    nrows = (t1 - t0) * 128
                    fts = []
                    for k in range(kchunks):
                        ft = ftp.tile([128, bsz * 128], f32, tag="ft")
                        nc.sync.dma_start(out=ft[:, :nrows],
                                          in_=featT[k, :, t0 * 128:t1 * 128])
                        fts.append(ft)
                    for t in range(t0, t1):
                        ro = (t - t0) * 128
                        psum1 = ps1.tile([p.nhid, 128], f32, tag="p1")
                        for k in range(kchunks):
                            nc.tensor.matmul(
                                psum1[:, :],
                                lhsT=w1sb[:, k * p.nhid:(k + 1) * p.nhid],
                                rhs=fts[k][:, ro:ro + 128],
                                start=(k == 0), stop=(k == kchunks - 1))
                        x1t = x1p.tile([p.nhid, 128], f32, tag="x1")
                        nc.scalar.activation(x1t[:, :], psum1[:, :],
                                             mybir.ActivationFunctionType.Relu,
                                             bias=b1sb[:, :1])
                        psum2 = ps2.tile([128, ncls], f32, tag="p2")
                        nc.tensor.matmul(psum2[:, :], lhsT=x1t[:, :], rhs=w2sb[:, :],
                                         start=True, stop=True)
                        nc.vector.tensor_tensor(
                            out=x_sb[:, t * EW:t * EW + ncls],
                            in0=psum2[:, :], in1=b2sb[:, :], op=mybir.AluOpType.add)
                        nc.vector.tensor_scalar_mul(
                            hp[:, t * ncls:(t + 1) * ncls],
                            x_sb[:, t * EW:t * EW + ncls], ALPHA)

            # ---- propagation ----
            def share_x():
                nc.sync.dma_start(
                    out=shard[:, :].rearrange('(t q) e -> q t e', q=128),
                    in_=x_sb[:, :].rearrange('p (t e) -> p t e', e=EW))
                nc.gpsimd.collective_compute(
                    "AllGather", mybir.AluOpType.bypass, replica_groups=rg,
                    ins=[shard[:, :]], outs=[xtable[:, :]])

            share_x()

            with tc.tile_pool(name="gout", bufs=6) as gp, \
                 tc.tile_pool(name="acc", bufs=2) as accp, \
                 tc.tile_pool(name="xadd", bufs=2) as xap:

                def do_group(gi):
                    t0, t1, sblocks, gcol0, gcols = gi
                    ngt = t1 - t0
                    g = gp.tile([128, max_gslots * ncls], bf16, tag="g")
                    for (s, c0, ncols_blk) in sblocks:
                        for cb in range(c0, c0 + ncols_blk, MAX_COLS):
                            ncols = min(MAX_COLS, c0 + ncols_blk - cb)
                            nidx = 128 * ncols
                            lo = (cb - gcol0) * ncls
                            _patched_dma_gather(
                                nc.gpsimd,
                                out_ap=g[:, lo:lo + ncols * ncls].rearrange(
                                    'p (s e) -> p s e', e=ncls),
                                in_ap=xtable[s * p.nsub:(s + 1) * p.nsub, :ncls],
                                idxs_ap=ixall[:, 8 * cb:8 * cb + nidx // 16],
                                num_idxs=nidx, num_idxs_reg=nidx,
                                elem_size=ncls, elem_step=EW,
                                queue_num=qn[0] % NQUEUES)
                            qn[0] += 1
                        # weighted multiply per sub-block (finer overlap)
                        lo = (c0 - gcol0) * ncls
                        nc.vector.tensor_tensor(
                            out=g[:, lo:lo + ncols_blk * ncls],
                            in0=g[:, lo:lo + ncols_blk * ncls],
                            in1=w_sb[:, c0:c0 + ncols_blk].to_broadcast(
                                [128, ncols_blk, ncls]),
                            op=mybir.AluOpType.mult)
                    # per-(tile, sub) reduce into acc[128, j, s, ncls] f32
                    acc = accp.tile([128, max_gt * 4 * ncls], f32, tag="acc")
                    for (s, c0, ncols) in sblocks:
                        cc = c0 - gcol0
                        for t in range(t0, t1):
                            l = int(L[t, s])
                            j = t - t0
                            nc.vector.tensor_reduce(
                                out=acc[:, (j * 4 + s) * ncls:(j * 4 + s + 1) * ncls],
                                in_=g[:, cc * ncls:(cc + l) * ncls].rearrange(
                                    'p (s e) -> p s e', e=ncls).transpose([0, 2, 1]),
                                axis=mybir.AxisListType.X, op=mybir.AluOpType.add)
                            cc += l
                    # combine 4 subs
                    xadd = xap.tile([128, max_gt * ncls], f32, tag="xa")
                    nc.vector.tensor_reduce(
                        out=xadd[:, :ngt * ncls],
                        in_=acc[:, :ngt * 4 * ncls].rearrange(
                            'p (j s e) -> p j s e', s=4, e=ncls).transpose([0, 1, 3, 2]),
                        axis=mybir.AxisListType.X, op=mybir.AluOpType.add)
                    # + alpha*h -> x_sb
                    nc.vector.tensor_tensor(
                        out=x_sb[:, :].rearrange('p (t e) -> p t e', e=EW)[:, t0:t1, :ncls],
                        in0=xadd[:, :ngt * ncls].rearrange('p (t e) -> p t e', e=ncls),
                        in1=hp[:, t0 * ncls:t1 * ncls].rearrange('p (t e) -> p t e', e=ncls),
                        op=mybir.AluOpType.add)

                for k in range(K_STEPS):
                    for gi in ginfo:
                        do_group(gi)
                    if k != K_STEPS - 1:
                        share_x()

            # ---- log_softmax ----
            with tc.tile_pool(name="smx", bufs=1) as smxp, \
                 tc.tile_pool(name="aggf", bufs=4) as aggp:
                xv16 = x_sb[:, :].rearrange('p (t e) -> p t e', e=EW)[:, :, :ncls]
                xf = smxp.tile([128, tiles * ncls], f32, tag="xf")
                nc.vector.tensor_copy(
                    out=xf[:, :].rearrange('p (t e) -> p t e', e=ncls), in_=xv16)
                xv = xf[:, :].rearrange('p (t e) -> p t e', e=ncls)
                sm = smxp.tile([128, tiles * ncls], f32, tag="sm")
                rmax = aggp.tile([128, tiles], f32, tag="aggf")
                nc.vector.tensor_reduce(out=rmax[:, :], in_=xv,
                                        axis=mybir.AxisListType.X, op=mybir.AluOpType.max)
                smv = sm[:, :tiles * ncls].rearrange('p (t e) -> p t e', e=ncls)
                nc.vector.tensor_tensor(
                    out=smv, in0=xv,
                    in1=rmax[:, :].to_broadcast([128, tiles, ncls]),
                    op=mybir.AluOpType.subtract)
                ex = smxp.tile([128, tiles * ncls], f32, tag="ex")
                nc.scalar.activation(ex[:, :], sm[:, :],
                                     mybir.ActivationFunctionType.Exp)
                ssum = aggp.tile([128, tiles], f32, tag="aggf")
                nc.vector.tensor_reduce(
                    out=ssum[:, :],
                    in_=ex[:, :].rearrange('p (t e) -> p t e', e=ncls),
                    axis=mybir.AxisListType.X, op=mybir.AluOpType.add)
                lsum = aggp.tile([128, tiles], f32, tag="aggf")
                nc.scalar.activation(lsum[:, :], ssum[:, :],
                                     mybir.ActivationFunctionType.Ln)
                nc.vector.tensor_tensor(
                    out=smv, in0=smv,
                    in1=lsum[:, :].to_broadcast([128, tiles, ncls]),
                    op=mybir.AluOpType.subtract)
                nc.sync.dma_start(
                    out=outp[:, :].rearrange('(t q) e -> q t e', q=128),
                    in_=sm[:, :].rearrange('p (t e) -> p t e', e=ncls))

    nc.compile()
    return nc


def run(features, edge_index, edge_values, W1, b1, W2, b2, trace=False):
    p = Plan()
    meta = preprocess(p, edge_index, edge_values)
    nc = build_kernel(p, meta)

    features = np.asarray(features, np.float32)
    W1 = np.asarray(W1, np.float32)
    b1 = np.asarray(b1, np.float32).reshape(-1)
    W2 = np.asarray(W2, np.float32)
    b2 = np.asarray(b2, np.float32).reshape(-1)
    kchunks = p.f_in // 128
    sortpos = meta["sortpos"]

    in_maps = []
    for c in range(NCORES):
        fpad = np.zeros((p.rp, p.f_in), np.float32)
        fpad[sortpos[c]] = features[c * p.rpc:(c + 1) * p.rpc]
        ft = np.ascontiguousarray(fpad.T).reshape(kchunks, 128, p.rp)
        in_maps.append({
            "featT": np.ascontiguousarray(ft),
            "W1p": np.ascontiguousarray(W1.reshape(kchunks, 128, p.nhid)
                                        .transpose(1, 0, 2).reshape(128, kchunks * p.nhid)),
            "b1p": np.ascontiguousarray(b1.reshape(p.nhid, 1)),
            "W2p": np.ascontiguousarray(W2),
            "b2p": np.ascontiguousarray(np.broadcast_to(b2, (128, p.ncls))),
            "idxp": np.ascontiguousarray(meta["IDXW"][c]),
            "wp": np.ascontiguousarray(meta["WG"][c]),
        })

    res = run_bass_kernel_spmd(nc, in_maps, list(range(NCORES)), trace=trace)

    out = np.empty((p.n, p.ncls), np.float32)
    for c in range(NCORES):
        o = res.results[c]["out"]
        out[c * p.rpc:(c + 1) * p.rpc] = o[sortpos[c]]
    return out, res


def kernel(**inputs):
    out, _ = run(inputs["features"], inputs["edge_index"], inputs["edge_values"],
                 inputs["W1"], inputs["b1"], inputs["W2"], inputs["b2"])
    return out


# revision 7
# speedup vs baseline: 1.0977x; 1.0044x over previous
"""APPNP (GNN message passing) on 8 Trainium2 NeuronCores — v2.

vs v1 (37.6ms -> ~17.9ms): degree-sorted tiles cut slot padding 1.91x->1.34x
(537k vs 763k gather idxs/step; desc-gen on GpSimd at ~3.4ns/idx is the wall),
tile-major positions, 543 full 8-column gather instrs/step (1024-idx HW cap),
idx table + weights persistent in SBUF (no per-step reloads), small tile-groups
(~140 slots) with 6-deep gather buffering to keep the gen stream fed, per-sub
weighted multiply + per-(tile,sub) f32 reduces + per-group sub-combine.
"""

import inspect
import math
import re
import sys

import numpy as np

if "/opt/trn_rl_repo" not in sys.path:
    sys.path.insert(0, "/opt/trn_rl_repo")

from concourse import bass, bacc, mybir  # noqa: E402
from concourse.tile import TileContext  # noqa: E402
from concourse.bass_utils import run_bass_kernel_spmd  # noqa: E402

N = 100000
F_IN = 512
NHID = 64
NCLS = 40
ALPHA = 0.1
K_STEPS = 10
NCORES = 8

EW = 128          # table row width in bf16 (256B stride)
NQUEUES = 4
MAX_COLS = 8      # per-gather column cap (1024 idxs/instr hardware limit)
GROUP_SLOTS = 105  # target slot columns per tile-group (SBUF budget)


def _make_patched_gather():
    src = inspect.getsource(bass.BassGpSimd.dma_gather)
    src = inspect.cleandoc("def dma_gather" + src.split("def dma_gather", 1)[1])
    src = re.sub(
        r"assert \(\s*elem_size_bytes > 0 and elem_size_bytes % 256 == 0\s*\)",
        "assert elem_size_bytes > 0",
        src,
    )
    assert "% 256 == 0" not in src.split("stride_bytes_256")[0]
    ns = vars(bass).copy()
    exec(src, ns)
    return ns["dma_gather"]


_patched_dma_gather = _make_patched_gather()


class Plan:
    def __init__(self):
        self.n = N
        self.ncls = NCLS
        self.nhid = NHID
        self.f_in = F_IN
        self.rpc = N // NCORES           # 12500
        self.tiles = math.ceil(self.rpc / 128)   # 98
        self.rp = self.tiles * 128       # 12544
        self.ntab = NCORES * self.rp     # 100352
        self.nsub = self.ntab // 4       # 25088 (= 2*rp, core-pair subtables)
        assert self.nsub <= 32768


def preprocess(p, edge_index, edge_values):
    dest = np.asarray(edge_index[0], np.int64)
    src = np.asarray(edge_index[1], np.int64)
    w = np.asarray(edge_values, np.float32) * (1.0 - ALPHA)

    core = dest // p.rpc
    local = dest - core * p.rpc
    sub = src // (2 * p.rpc)                     # source's core-pair = subtable

    degs = np.zeros((NCORES, p.rpc, 4), np.int64)
    np.add.at(degs, (core, local, sub), 1)
    deg = degs.sum(axis=2)
    mx = degs.max(axis=1 + 1)                    # [NCORES, rpc] max over subs

    sortpos = np.empty((NCORES, p.rpc), np.int64)
    for c in range(NCORES):
        dv = degs[c]
        order = np.lexsort((dv[:, 3], dv[:, 2], dv[:, 1], dv[:, 0],
                            dv[:, 3] // 2, dv[:, 2] // 2, dv[:, 1] // 2,
                            dv[:, 0] // 2, -mx[c]))
        sortpos[c, order] = np.arange(p.rpc)

    # tile-major positions: pos = t*128 + part
    s_core = src // p.rpc
    pos_s = sortpos[s_core, src - s_core * p.rpc]
    idx16 = (s_core % 2) * p.rp + pos_s          # row within subtable

    pos_d = sortpos[core, local]
    dt = pos_d // 128
    dp = pos_d - dt * 128

    counts = np.zeros((NCORES, p.tiles, 4, 128), np.int64)
    np.add.at(counts, (core, dt, sub, dp), 1)
    L = np.maximum(counts.max(axis=(0, 3)), 1)   # [tiles, 4]

    # groups: consecutive tiles while total slots stay <= GROUP_SLOTS
    groups = []
    t0 = 0
    while t0 < p.tiles:
        t1 = t0 + 1
        while t1 < p.tiles and L[t0:t1 + 1].sum() <= GROUP_SLOTS:
            t1 += 1
        groups.append((t0, t1))
        t0 = t1

    # column layout: [g0: s0 blocks t0..t1 | s1 | s2 | s3][g1: ...]
    col_off = np.zeros((p.tiles, 4), np.int64)
    ginfo = []                                   # per group: (t0,t1,[ (s, c0, ncols) x4 ], gcol0, gcols)
    cur = 0
    for (t0, t1) in groups:
        gcol0 = cur
        sblocks = []
        for s in range(4):
            c0 = cur
            for t in range(t0, t1):
                col_off[t, s] = cur
                cur += int(L[t, s])
            sblocks.append((s, c0, cur - c0))
        ginfo.append((t0, t1, sblocks, gcol0, cur - gcol0))
    total_slots = cur

    # slot rank within (core, tile, sub, part); secondary sort by table row
    grp = (((core * p.tiles + dt) * 4 + sub) * 128 + dp)
    sort_idx = np.lexsort((idx16, grp))
    grp_sorted = grp[sort_idx]
    starts = np.r_[0, np.flatnonzero(np.diff(grp_sorted)) + 1]
    gs = np.zeros(len(grp_sorted), np.int64)
    gs[starts] = starts
    gs = np.maximum.accumulate(gs)
    rank = np.empty(len(grp_sorted), np.int64)
    rank[sort_idx] = np.arange(len(grp_sorted)) - gs

    IDX = np.zeros((NCORES, 128, total_slots), np.int64)
    WG = np.zeros((NCORES, 128, total_slots), np.float32)
    colv = col_off[dt, sub] + rank
    IDX[core, dp, colv] = idx16
    WG[core, dp, colv] = w

    # wrapped idx image: instructions are <=MAX_COLS-column chunks of each
    # (group, sub) block; wrap each chunk [16, 8*cols] and replicate x8
    wrap_cols = 8 * total_slots
    IDXW = np.zeros((NCORES, 128, wrap_cols), np.int16)
    for (t0, t1, sblocks, gcol0, gcols) in ginfo:
        for (s, c0, ncols_blk) in sblocks:
            for cb in range(c0, c0 + ncols_blk, MAX_COLS):
                ncols = min(MAX_COLS, c0 + ncols_blk - cb)
                nidx = 128 * ncols
                ncw = nidx // 16
                ii = np.arange(nidx)
                rr, cc = ii % 16, ii // 16
                woff = 8 * cb
                for c in range(NCORES):
                    flat = IDX[c, :, cb:cb + ncols].T.reshape(-1)
                    w16 = np.zeros((16, ncw), np.int16)
                    w16[rr, cc] = flat.astype(np.int16)
                    IDXW[c, :, woff:woff + ncw] = np.tile(w16, (8, 1))

    import ml_dtypes
    WG = WG.astype(ml_dtypes.bfloat16)
    return dict(sortpos=sortpos, total_slots=total_slots, L=L, ginfo=ginfo,
                IDXW=IDXW, WG=WG)


def build_kernel(p, meta):
    tiles = p.tiles
    total_slots = int(meta["total_slots"])
    L = meta["L"]
    ginfo = meta["ginfo"]
    kchunks = p.f_in // 128
    ncls = p.ncls
    max_gslots = max(g[4] for g in ginfo)
    max_gt = max(g[1] - g[0] for g in ginfo)

    nc = bacc.Bacc("TRN2", target_bir_lowering=False, debug=False,
                   num_devices=NCORES, num_swdge_queues=NQUEUES)

    f32 = mybir.dt.float32
    bf16 = mybir.dt.bfloat16
    i16 = mybir.dt.int16
    featT = nc.declare_dram_parameter("featT", [kchunks, 128, p.rp], f32, isOutput=False)
    W1p = nc.declare_dram_parameter("W1p", [128, kchunks * p.nhid], f32, isOutput=False)
    b1p = nc.declare_dram_parameter("b1p", [p.nhid, 1], f32, isOutput=False)
    W2p = nc.declare_dram_parameter("W2p", [p.nhid, ncls], f32, isOutput=False)
    b2p = nc.declare_dram_parameter("b2p", [128, ncls], f32, isOutput=False)
    idxp = nc.declare_dram_parameter("idxp", [128, 8 * total_slots], i16, isOutput=False)
    wp = nc.declare_dram_parameter("wp", [128, total_slots], bf16, isOutput=False)
    outp = nc.declare_dram_parameter("out", [p.rp, ncls], f32, isOutput=True)

    shard = nc.dram_tensor("shard", [p.rp, EW], bf16)
    xtable = nc.dram_tensor("xtable", [p.ntab, EW], bf16, addr_space="Shared")
    rg = [list(range(NCORES))]
    qn = [0]

    with TileContext(nc) as tc, \
         nc.allow_low_precision(reason="bf16 propagation validated on baseline (rel 2.6e-3)"):
        with tc.tile_pool(name="persist", bufs=1) as pp:
            x_sb = pp.tile([128, tiles * EW], bf16)
            hp = pp.tile([128, tiles * ncls], bf16)
            w_sb = pp.tile([128, total_slots], bf16)
            ixall = pp.tile([128, 8 * total_slots], i16)
            nc.vector.memset(x_sb[:, :], 0.0)
            nc.sync.dma_start(out=w_sb[:, :], in_=wp[:, :])
            nc.sync.dma_start(out=ixall[:, :], in_=idxp[:, :])

            # ---- fc phase ----
            with tc.tile_pool(name="psum1", bufs=4, space="PSUM") as ps1, \
                 tc.tile_pool(name="psum2", bufs=4, space="PSUM") as ps2, \
                 tc.tile_pool(name="fcw", bufs=1) as fcw, \
                 tc.tile_pool(name="ft", bufs=4) as ftp, \
                 tc.tile_pool(name="x1", bufs=4) as x1p:
                w1sb = fcw.tile([128, kchunks * p.nhid], f32)
                nc.sync.dma_start(out=w1sb[:, :], in_=W1p[:, :])
                w2sb = fcw.tile([p.nhid, ncls], f32)
                nc.sync.dma_start(out=w2sb[:, :], in_=W2p[:, :])
                b1sb = fcw.tile([p.nhid, 1], f32)
                nc.sync.dma_start(out=b1sb[:, :], in_=b1p[:, :])
                b2sb = fcw.tile([128, ncls], f32)
                nc.sync.dma_start(out=b2sb[:, :], in_=b2p[:, :])

                nbatch = min(14, tiles)
                bsz = math.ceil(tiles / nbatch)
                for b in range(nbatch):
                    t0 = b * bsz
                    t1 = min(tiles, t0 + bsz)
                    if t0 >= t1:
                        continue
                    nrows = (t1 - t0) * 128
                    fts = []
                    for k in range(kchunks):
                        ft = ftp.tile([128, bsz * 128], f32, tag="ft")
                        nc.sync.dma_start(out=ft[:, :nrows],
                                          in_=featT[k, :, t0 * 128:t1 * 128])
                        fts.append(ft)
                    for t in range(t0, t1):
                        ro = (t - t0) * 128
                        psum1 = ps1.tile([p.nhid, 128], f32, tag="p1")
                        for k in range(kchunks):
                            nc.tensor.matmul(
                                psum1[:, :],
                                lhsT=w1sb[:, k * p.nhid:(k + 1) * p.nhid],
                                rhs=fts[k][:, ro:ro + 128],
                                start=(k == 0), stop=(k == kchunks - 1))
                        x1t = x1p.tile([p.nhid, 128], f32, tag="x1")
                        nc.scalar.activation(x1t[:, :], psum1[:, :],
                                             mybir.ActivationFunctionType.Relu,
                                             bias=b1sb[:, :1])
                        psum2 = ps2.tile([128, ncls], f32, tag="p2")
                        nc.tensor.matmul(psum2[:, :], lhsT=x1t[:, :], rhs=w2sb[:, :],
                                         start=True, stop=True)
                        nc.vector.tensor_tensor(
                            out=x_sb[:, t * EW:t * EW + ncls],
                            in0=psum2[:, :], in1=b2sb[:, :], op=mybir.AluOpType.add)
                        nc.vector.tensor_scalar_mul(
                            hp[:, t * ncls:(t + 1) * ncls],
                            x_sb[:, t * EW:t * EW + ncls], ALPHA)

            # ---- propagation ----
            def share_x():
                nc.sync.dma_start(
                    out=shard[:, :].rearrange('(t q) e -> q t e', q=128),
                    in_=x_sb[:, :].rearrange('p (t e) -> p t e', e=EW))
                nc.gpsimd.collective_compute(
                    "AllGather", mybir.AluOpType.bypass, replica_groups=rg,
                    ins=[shard[:, :]], outs=[xtable[:, :]])

            share_x()

            with tc.tile_pool(name="gout", bufs=8) as gp, \
                 tc.tile_pool(name="acc", bufs=2) as accp, \
                 tc.tile_pool(name="xadd", bufs=2) as xap:

                def do_group(gi):
                    t0, t1, sblocks, gcol0, gcols = gi
                    ngt = t1 - t0
                    g = gp.tile([128, max_gslots * ncls], bf16, tag="g")
                    for (s, c0, ncols_blk) in sblocks:
                        for cb in range(c0, c0 + ncols_blk, MAX_COLS):
                            ncols = min(MAX_COLS, c0 + ncols_blk - cb)
                            nidx = 128 * ncols
                            lo = (cb - gcol0) * ncls
                            _patched_dma_gather(
                                nc.gpsimd,
                                out_ap=g[:, lo:lo + ncols * ncls].rearrange(
                                    'p (s e) -> p s e', e=ncls),
                                in_ap=xtable[s * p.nsub:(s + 1) * p.nsub, :ncls],
                                idxs_ap=ixall[:, 8 * cb:8 * cb + nidx // 16],
                                num_idxs=nidx, num_idxs_reg=nidx,
                                elem_size=ncls, elem_step=EW,
                                queue_num=qn[0] % NQUEUES)
                            qn[0] += 1
                        # weighted multiply per sub-block (finer overlap)
                        lo = (c0 - gcol0) * ncls
                        nc.vector.tensor_tensor(
                            out=g[:, lo:lo + ncols_blk * ncls],
                            in0=g[:, lo:lo + ncols_blk * ncls],
                            in1=w_sb[:, c0:c0 + ncols_blk].to_broadcast(
                                [128, ncols_blk, ncls]),
                            op=mybir.AluOpType.mult)
                    # per-(tile, sub) reduce into acc[128, j, s, ncls] f32
                    acc = accp.tile([128, max_gt * 4 * ncls], f32, tag="acc")
                    for (s, c0, ncols) in sblocks:
                        cc = c0 - gcol0
                        for t in range(t0, t1):
                            l = int(L[t, s])
                            j = t - t0
                            nc.vector.tensor_reduce(
                                out=acc[:, (j * 4 + s) * ncls:(j * 4 + s + 1) * ncls],
                                in_=g[:, cc * ncls:(cc + l) * ncls].rearrange(
                                    'p (s e) -> p s e', e=ncls).transpose([0, 2, 1]),
                                axis=mybir.AxisListType.X, op=mybir.AluOpType.add)
                            cc += l
                    # combine 4 subs
                    xadd = xap.tile([128, max_gt * ncls], f32, tag="xa")
                    nc.vector.tensor_reduce(
                        out=xadd[:, :ngt * ncls],
                        in_=acc[:, :ngt * 4 * ncls].rearrange(
                            'p (j s e) -> p j s e', s=4, e=ncls).transpose([0, 1, 3, 2]),
                        axis=mybir.AxisListType.X, op=mybir.AluOpType.add)
                    # + alpha*h -> x_sb
                    nc.vector.tensor_tensor(
                        out=x_sb[:, :].rearrange('p (t e) -> p t e', e=EW)[:, t0:t1, :ncls],
                        in0=xadd[:, :ngt * ncls].rearrange('p (t e) -> p t e', e=ncls),
                        in1=hp[:, t0 * ncls:t1 * ncls].rearrange('p (t e) -> p t e', e=ncls),
                        op=mybir.AluOpType.add)

                for k in range(K_STEPS):
                    for gi in ginfo:
                        do_group(gi)
                    if k != K_STEPS - 1:
                        share_x()

            # ---- log_softmax ----
            with tc.tile_pool(name="smx", bufs=1) as smxp, \
                 tc.tile_pool(name="aggf", bufs=4) as aggp:
                xv16 = x_sb[:, :].rearrange('p (t e) -> p t e', e=EW)[:, :, :ncls]
                xf = smxp.tile([128, tiles * ncls], f32, tag="xf")
                nc.vector.tensor_copy(
                    out=xf[:, :].rearrange('p (t e) -> p t e', e=ncls), in_=xv16)
                xv = xf[:, :].rearrange('p (t e) -> p t e', e=ncls)
                sm = smxp.tile([128, tiles * ncls], f32, tag="sm")
                rmax = aggp.tile([128, tiles], f32, tag="aggf")
                nc.vector.tensor_reduce(out=rmax[:, :], in_=xv,
                                        axis=mybir.AxisListType.X, op=mybir.AluOpType.max)
                smv = sm[:, :tiles * ncls].rearrange('p (t e) -> p t e', e=ncls)
                nc.vector.tensor_tensor(
                    out=smv, in0=xv,
                    in1=rmax[:, :].to_broadcast([128, tiles, ncls]),
                    op=mybir.AluOpType.subtract)
                ex = smxp.tile([128, tiles * ncls], f32, tag="ex")
                nc.scalar.activation(ex[:, :], sm[:, :],
                                     mybir.ActivationFunctionType.Exp)
                ssum = aggp.tile([128, tiles], f32, tag="aggf")
                nc.vector.tensor_reduce(
                    out=ssum[:, :],
                    in_=ex[:, :].rearrange('p (t e) -> p t e', e=ncls),
                    axis=mybir.AxisListType.X, op=mybir.AluOpType.add)
                lsum = aggp.tile([128, tiles], f32, tag="aggf")
                nc.scalar.activation(lsum[:, :], ssum[:, :],
                                     mybir.ActivationFunctionType.Ln)
                nc.vector.tensor_tensor(
                    out=smv, in0=smv,
                    in1=lsum[:, :].to_broadcast([128, tiles, ncls]),
                    op=mybir.AluOpType.subtract)
                nc.sync.dma_start(
                    out=outp[:, :].rearrange('(t q) e -> q t e', q=128),
                    in_=sm[:, :].rearrange('p (t e) -> p t e', e=ncls))

    nc.compile()
    return nc


def run(features, edge_index, edge_values, W1, b1, W2, b2, trace=False):
    p = Plan()
    meta = preprocess(p, edge_index, edge_values)
    nc = build_kernel(p, meta)

    features = np.asarray(features, np.float32)
    W1 = np.asarray(W1, np.float32)
    b1 = np.asarray(b1, np.float32).reshape(-1)
    W2 = np.asarray(W2, np.float32)
    b2 = np.asarray(b2, np.float32).reshape(-1)
    kchunks = p.f_in // 128
    sortpos = meta["sortpos"]

    in_maps = []
    for c in range(NCORES):
        fpad = np.zeros((p.rp, p.f_in), np.float32)
        fpad[sortpos[c]] = features[c * p.rpc:(c + 1) * p.rpc]
        ft = np.ascontiguousarray(fpad.T).reshape(kchunks, 128, p.rp)
        in_maps.append({
            "featT": np.ascontiguousarray(ft),
            "W1p": np.ascontiguousarray(W1.reshape(kchunks, 128, p.nhid)
                                        .transpose(1, 0, 2).reshape(128, kchunks * p.nhid)),
            "b1p": np.ascontiguousarray(b1.reshape(p.nhid, 1)),
            "W2p": np.ascontiguousarray(W2),
            "b2p": np.ascontiguousarray(np.broadcast_to(b2, (128, p.ncls))),
            "idxp": np.ascontiguousarray(meta["IDXW"][c]),
            "wp": np.ascontiguousarray(meta["WG"][c]),
        })

    res = run_bass_kernel_spmd(nc, in_maps, list(range(NCORES)), trace=trace)

    out = np.empty((p.n, p.ncls), np.float32)
    for c in range(NCORES):
        o = res.results[c]["out"]
        out[c * p.rpc:(c + 1) * p.rpc] = o[sortpos[c]]
    return out, res


def kernel(**inputs):
    out, _ = run(inputs["features"], inputs["edge_index"], inputs["edge_values"],
                 inputs["W1"], inputs["b1"], inputs["W2"], inputs["b2"])
    return out


# revision 8
# speedup vs baseline: 1.1009x; 1.0029x over previous
"""APPNP (GNN message passing) on 8 Trainium2 NeuronCores — v2.

vs v1 (37.6ms -> ~17.9ms): degree-sorted tiles cut slot padding 1.91x->1.34x
(537k vs 763k gather idxs/step; desc-gen on GpSimd at ~3.4ns/idx is the wall),
tile-major positions, 543 full 8-column gather instrs/step (1024-idx HW cap),
idx table + weights persistent in SBUF (no per-step reloads), small tile-groups
(~96 slots) with 10-deep gather buffering to keep the gen stream fed, per-sub
weighted multiply + per-(tile,sub) f32 reduces + per-group sub-combine.
"""

import inspect
import math
import re
import sys

import numpy as np

if "/opt/trn_rl_repo" not in sys.path:
    sys.path.insert(0, "/opt/trn_rl_repo")

from concourse import bass, bacc, mybir  # noqa: E402
from concourse.tile import TileContext  # noqa: E402
from concourse.bass_utils import run_bass_kernel_spmd  # noqa: E402

N = 100000
F_IN = 512
NHID = 64
NCLS = 40
ALPHA = 0.1
K_STEPS = 10
NCORES = 8

EW = 128          # table row width in bf16 (256B stride)
NQUEUES = 4
MAX_COLS = 8      # per-gather column cap (1024 idxs/instr hardware limit)
GROUP_SLOTS = 96  # target slot columns per tile-group (SBUF budget)


def _make_patched_gather():
    src = inspect.getsource(bass.BassGpSimd.dma_gather)
    src = inspect.cleandoc("def dma_gather" + src.split("def dma_gather", 1)[1])
    src = re.sub(
        r"assert \(\s*elem_size_bytes > 0 and elem_size_bytes % 256 == 0\s*\)",
        "assert elem_size_bytes > 0",
        src,
    )
    assert "% 256 == 0" not in src.split("stride_bytes_256")[0]
    ns = vars(bass).copy()
    exec(src, ns)
    return ns["dma_gather"]


_patched_dma_gather = _make_patched_gather()


class Plan:
    def __init__(self):
        self.n = N
        self.ncls = NCLS
        self.nhid = NHID
        self.f_in = F_IN
        self.rpc = N // NCORES           # 12500
        self.tiles = math.ceil(self.rpc / 128)   # 98
        self.rp = self.tiles * 128       # 12544
        self.ntab = NCORES * self.rp     # 100352
        self.nsub = self.ntab // 4       # 25088 (= 2*rp, core-pair subtables)
        assert self.nsub <= 32768


def preprocess(p, edge_index, edge_values):
    dest = np.asarray(edge_index[0], np.int64)
    src = np.asarray(edge_index[1], np.int64)
    w = np.asarray(edge_values, np.float32) * (1.0 - ALPHA)

    core = dest // p.rpc
    local = dest - core * p.rpc
    sub = src // (2 * p.rpc)                     # source's core-pair = subtable

    degs = np.zeros((NCORES, p.rpc, 4), np.int64)
    np.add.at(degs, (core, local, sub), 1)
    deg = degs.sum(axis=2)
    mx = degs.max(axis=1 + 1)                    # [NCORES, rpc] max over subs

    sortpos = np.empty((NCORES, p.rpc), np.int64)
    for c in range(NCORES):
        dv = degs[c]
        order = np.lexsort((dv[:, 3], dv[:, 2], dv[:, 1], dv[:, 0],
                            dv[:, 3] // 2, dv[:, 2] // 2, dv[:, 1] // 2,
                            dv[:, 0] // 2, -mx[c]))
        sortpos[c, order] = np.arange(p.rpc)

    # tile-major positions: pos = t*128 + part
    s_core = src // p.rpc
    pos_s = sortpos[s_core, src - s_core * p.rpc]
    idx16 = (s_core % 2) * p.rp + pos_s          # row within subtable

    pos_d = sortpos[core, local]
    dt = pos_d // 128
    dp = pos_d - dt * 128

    counts = np.zeros((NCORES, p.tiles, 4, 128), np.int64)
    np.add.at(counts, (core, dt, sub, dp), 1)
    L = np.maximum(counts.max(axis=(0, 3)), 1)   # [tiles, 4]

    # groups: consecutive tiles while total slots stay <= GROUP_SLOTS
    groups = []
    t0 = 0
    while t0 < p.tiles:
        t1 = t0 + 1
        while t1 < p.tiles and L[t0:t1 + 1].sum() <= GROUP_SLOTS:
            t1 += 1
        groups.append((t0, t1))
        t0 = t1

    # column layout: [g0: s0 blocks t0..t1 | s1 | s2 | s3][g1: ...]
    col_off = np.zeros((p.tiles, 4), np.int64)
    ginfo = []                                   # per group: (t0,t1,[ (s, c0, ncols) x4 ], gcol0, gcols)
    cur = 0
    for (t0, t1) in groups:
        gcol0 = cur
        sblocks = []
        for s in range(4):
            c0 = cur
            for t in range(t0, t1):
                col_off[t, s] = cur
                cur += int(L[t, s])
            sblocks.append((s, c0, cur - c0))
        ginfo.append((t0, t1, sblocks, gcol0, cur - gcol0))
    total_slots = cur

    # slot rank within (core, tile, sub, part); secondary sort by table row
    grp = (((core * p.tiles + dt) * 4 + sub) * 128 + dp)
    sort_idx = np.lexsort((idx16, grp))
    grp_sorted = grp[sort_idx]
    starts = np.r_[0, np.flatnonzero(np.diff(grp_sorted)) + 1]
    gs = np.zeros(len(grp_sorted), np.int64)
    gs[starts] = starts
    gs = np.maximum.accumulate(gs)
    rank = np.empty(len(grp_sorted), np.int64)
    rank[sort_idx] = np.arange(len(grp_sorted)) - gs

    IDX = np.zeros((NCORES, 128, total_slots), np.int64)
    WG = np.zeros((NCORES, 128, total_slots), np.float32)
    colv = col_off[dt, sub] + rank
    IDX[core, dp, colv] = idx16
    WG[core, dp, colv] = w

    # wrapped idx image: instructions are <=MAX_COLS-column chunks of each
    # (group, sub) block; wrap each chunk [16, 8*cols] and replicate x8
    wrap_cols = 8 * total_slots
    IDXW = np.zeros((NCORES, 128, wrap_cols), np.int16)
    for (t0, t1, sblocks, gcol0, gcols) in ginfo:
        for (s, c0, ncols_blk) in sblocks:
            for cb in range(c0, c0 + ncols_blk, MAX_COLS):
                ncols = min(MAX_COLS, c0 + ncols_blk - cb)
                nidx = 128 * ncols
                ncw = nidx // 16
                ii = np.arange(nidx)
                rr, cc = ii % 16, ii // 16
                woff = 8 * cb
                for c in range(NCORES):
                    flat = IDX[c, :, cb:cb + ncols].T.reshape(-1)
                    w16 = np.zeros((16, ncw), np.int16)
                    w16[rr, cc] = flat.astype(np.int16)
                    IDXW[c, :, woff:woff + ncw] = np.tile(w16, (8, 1))

    import ml_dtypes
    WG = WG.astype(ml_dtypes.bfloat16)
    return dict(sortpos=sortpos, total_slots=total_slots, L=L, ginfo=ginfo,
                IDXW=IDXW, WG=WG)


def build_kernel(p, meta):
    tiles = p.tiles
    total_slots = int(meta["total_slots"])
    L = meta["L"]
    ginfo = meta["ginfo"]
    kchunks = p.f_in // 128
    ncls = p.ncls
    max_gslots = max(g[4] for g in ginfo)
    max_gt = max(g[1] - g[0] for g in ginfo)

    nc = bacc.Bacc("TRN2", target_bir_lowering=False, debug=False,
                   num_devices=NCORES, num_swdge_queues=NQUEUES)

    f32 = mybir.dt.float32
    bf16 = mybir.dt.bfloat16
    i16 = mybir.dt.int16
    featT = nc.declare_dram_parameter("featT", [kchunks, 128, p.rp], f32, isOutput=False)
    W1p = nc.declare_dram_parameter("W1p", [128, kchunks * p.nhid], f32, isOutput=False)
    b1p = nc.declare_dram_parameter("b1p", [p.nhid, 1], f32, isOutput=False)
    W2p = nc.declare_dram_parameter("W2p", [p.nhid, ncls], f32, isOutput=False)
    b2p = nc.declare_dram_parameter("b2p", [128, ncls], f32, isOutput=False)
    idxp = nc.declare_dram_parameter("idxp", [128, 8 * total_slots], i16, isOutput=False)
    wp = nc.declare_dram_parameter("wp", [128, total_slots], bf16, isOutput=False)
    outp = nc.declare_dram_parameter("out", [p.rp, ncls], f32, isOutput=True)

    shard = nc.dram_tensor("shard", [p.rp, EW], bf16)
    xtable = nc.dram_tensor("xtable", [p.ntab, EW], bf16, addr_space="Shared")
    rg = [list(range(NCORES))]
    qn = [0]

    with TileContext(nc) as tc, \
         nc.allow_low_precision(reason="bf16 propagation validated on baseline (rel 2.6e-3)"):
        with tc.tile_pool(name="persist", bufs=1) as pp:
            x_sb = pp.tile([128, tiles * EW], bf16)
            hp = pp.tile([128, tiles * ncls], bf16)
            w_sb = pp.tile([128, total_slots], bf16)
            ixall = pp.tile([128, 8 * total_slots], i16)
            nc.vector.memset(x_sb[:, :], 0.0)
            nc.sync.dma_start(out=w_sb[:, :], in_=wp[:, :])
            nc.sync.dma_start(out=ixall[:, :], in_=idxp[:, :])

            # ---- fc phase ----
            with tc.tile_pool(name="psum1", bufs=4, space="PSUM") as ps1, \
                 tc.tile_pool(name="psum2", bufs=4, space="PSUM") as ps2, \
                 tc.tile_pool(name="fcw", bufs=1) as fcw, \
                 tc.tile_pool(name="ft", bufs=4) as ftp, \
                 tc.tile_pool(name="x1", bufs=4) as x1p:
                w1sb = fcw.tile([128, kchunks * p.nhid], f32)
                nc.sync.dma_start(out=w1sb[:, :], in_=W1p[:, :])
                w2sb = fcw.tile([p.nhid, ncls], f32)
                nc.sync.dma_start(out=w2sb[:, :], in_=W2p[:, :])
                b1sb = fcw.tile([p.nhid, 1], f32)
                nc.sync.dma_start(out=b1sb[:, :], in_=b1p[:, :])
                b2sb = fcw.tile([128, ncls], f32)
                nc.sync.dma_start(out=b2sb[:, :], in_=b2p[:, :])

                nbatch = min(14, tiles)
                bsz = math.ceil(tiles / nbatch)
                for b in range(nbatch):
                    t0 = b * bsz
                    t1 = min(tiles, t0 + bsz)
                    if t0 >= t1:
                        continue
                    nrows = (t1 - t0) * 128
                    fts = []
                    for k in range(kchunks):
                        ft = ftp.tile([128, bsz * 128], f32, tag="ft")
                        nc.sync.dma_start(out=ft[:, :nrows],
                                          in_=featT[k, :, t0 * 128:t1 * 128])
                        fts.append(ft)
                    for t in range(t0, t1):
                        ro = (t - t0) * 128
                        psum1 = ps1.tile([p.nhid, 128], f32, tag="p1")
                        for k in range(kchunks):
                            nc.tensor.matmul(
                                psum1[:, :],
                                lhsT=w1sb[:, k * p.nhid:(k + 1) * p.nhid],
                                rhs=fts[k][:, ro:ro + 128],
                                start=(k == 0), stop=(k == kchunks - 1))
                        x1t = x1p.tile([p.nhid, 128], f32, tag="x1")
                        nc.scalar.activation(x1t[:, :], psum1[:, :],
                                             mybir.ActivationFunctionType.Relu,
                                             bias=b1sb[:, :1])
                        psum2 = ps2.tile([128, ncls], f32, tag="p2")
                        nc.tensor.matmul(psum2[:, :], lhsT=x1t[:, :], rhs=w2sb[:, :],
                                         start=True, stop=True)
                        nc.vector.tensor_tensor(
                            out=x_sb[:, t * EW:t * EW + ncls],
                            in0=psum2[:, :], in1=b2sb[:, :], op=mybir.AluOpType.add)
                        nc.vector.tensor_scalar_mul(
                            hp[:, t * ncls:(t + 1) * ncls],
                            x_sb[:, t * EW:t * EW + ncls], ALPHA)

            # ---- propagation ----
            def share_x():
                nc.sync.dma_start(
                    out=shard[:, :].rearrange('(t q) e -> q t e', q=128),
                    in_=x_sb[:, :].rearrange('p (t e) -> p t e', e=EW))
                nc.gpsimd.collective_compute(
                    "AllGather", mybir.AluOpType.bypass, replica_groups=rg,
                    ins=[shard[:, :]], outs=[xtable[:, :]])

            share_x()

            with tc.tile_pool(name="gout", bufs=10) as gp, \
                 tc.tile_pool(name="acc", bufs=2) as accp, \
                 tc.tile_pool(name="xadd", bufs=2) as xap:

                def do_group(gi):
                    t0, t1, sblocks, gcol0, gcols = gi
                    ngt = t1 - t0
                    g = gp.tile([128, max_gslots * ncls], bf16, tag="g")
                    for (s, c0, ncols_blk) in sblocks:
                        for cb in range(c0, c0 + ncols_blk, MAX_COLS):
                            ncols = min(MAX_COLS, c0 + ncols_blk - cb)
                            nidx = 128 * ncols
                            lo = (cb - gcol0) * ncls
                            _patched_dma_gather(
                                nc.gpsimd,
                                out_ap=g[:, lo:lo + ncols * ncls].rearrange(
                                    'p (s e) -> p s e', e=ncls),
                                in_ap=xtable[s * p.nsub:(s + 1) * p.nsub, :ncls],
                                idxs_ap=ixall[:, 8 * cb:8 * cb + nidx // 16],
                                num_idxs=nidx, num_idxs_reg=nidx,
                                elem_size=ncls, elem_step=EW,
                                queue_num=qn[0] % NQUEUES)
                            qn[0] += 1
                        # weighted multiply per sub-block (finer overlap)
                        lo = (c0 - gcol0) * ncls
                        nc.vector.tensor_tensor(
                            out=g[:, lo:lo + ncols_blk * ncls],
                            in0=g[:, lo:lo + ncols_blk * ncls],
                            in1=w_sb[:, c0:c0 + ncols_blk].to_broadcast(
                                [128, ncols_blk, ncls]),
                            op=mybir.AluOpType.mult)
                    # per-(tile, sub) reduce into acc[128, j, s, ncls] f32
                    acc = accp.tile([128, max_gt * 4 * ncls], f32, tag="acc")
                    for (s, c0, ncols) in sblocks:
                        cc = c0 - gcol0
                        for t in range(t0, t1):
                            l = int(L[t, s])
                            j = t - t0
                            nc.vector.tensor_reduce(
                                out=acc[:, (j * 4 + s) * ncls:(j * 4 + s + 1) * ncls],
                                in_=g[:, cc * ncls:(cc + l) * ncls].rearrange(
                                    'p (s e) -> p s e', e=ncls).transpose([0, 2, 1]),
                                axis=mybir.AxisListType.X, op=mybir.AluOpType.add)
                            cc += l
                    # combine 4 subs
                    xadd = xap.tile([128, max_gt * ncls], f32, tag="xa")
                    nc.vector.tensor_reduce(
                        out=xadd[:, :ngt * ncls],
                        in_=acc[:, :ngt * 4 * ncls].rearrange(
                            'p (j s e) -> p j s e', s=4, e=ncls).transpose([0, 1, 3, 2]),
                        axis=mybir.AxisListType.X, op=mybir.AluOpType.add)
                    # + alpha*h -> x_sb
                    nc.vector.tensor_tensor(
                        out=x_sb[:, :].rearrange('p (t e) -> p t e', e=EW)[:, t0:t1, :ncls],
                        in0=xadd[:, :ngt * ncls].rearrange('p (t e) -> p t e', e=ncls),
                        in1=hp[:, t0 * ncls:t1 * ncls].rearrange('p (t e) -> p t e', e=ncls),
                        op=mybir.AluOpType.add)

                for k in range(K_STEPS):
                    for gi in ginfo:
                        do_group(gi)
                    if k != K_STEPS - 1:
                        share_x()

            # ---- log_softmax ----
            with tc.tile_pool(name="smx", bufs=1) as smxp, \
                 tc.tile_pool(name="aggf", bufs=4) as aggp:
                xv16 = x_sb[:, :].rearrange('p (t e) -> p t e', e=EW)[:, :, :ncls]
                xf = smxp.tile([128, tiles * ncls], f32, tag="xf")
                nc.vector.tensor_copy(
                    out=xf[:, :].rearrange('p (t e) -> p t e', e=ncls), in_=xv16)
                xv = xf[:, :].rearrange('p (t e) -> p t e', e=ncls)
                sm = smxp.tile([128, tiles * ncls], f32, tag="sm")
                rmax = aggp.tile([128, tiles], f32, tag="aggf")
                nc.vector.tensor_reduce(out=rmax[:, :], in_=xv,
                                        axis=mybir.AxisListType.X, op=mybir.AluOpType.max)
                smv = sm[:, :tiles * ncls].rearrange('p (t e) -> p t e', e=ncls)
                nc.vector.tensor_tensor(
                    out=smv, in0=xv,
                    in1=rmax[:, :].to_broadcast([128, tiles, ncls]),
                    op=mybir.AluOpType.subtract)
                ex = smxp.tile([128, tiles * ncls], f32, tag="ex")
                nc.scalar.activation(ex[:, :], sm[:, :],
                                     mybir.ActivationFunctionType.Exp)
                ssum = aggp.tile([128, tiles], f32, tag="aggf")
                nc.vector.tensor_reduce(
                    out=ssum[:, :],
                    in_=ex[:, :].rearrange('p (t e) -> p t e', e=ncls),
                    axis=mybir.AxisListType.X, op=mybir.AluOpType.add)
                lsum = aggp.tile([128, tiles], f32, tag="aggf")
                nc.scalar.activation(lsum[:, :], ssum[:, :],
                                     mybir.ActivationFunctionType.Ln)
                nc.vector.tensor_tensor(
                    out=smv, in0=smv,
                    in1=lsum[:, :].to_broadcast([128, tiles, ncls]),
                    op=mybir.AluOpType.subtract)
                nc.sync.dma_start(
                    out=outp[:, :].rearrange('(t q) e -> q t e', q=128),
                    in_=sm[:, :].rearrange('p (t e) -> p t e', e=ncls))

    nc.compile()
    return nc


def run(features, edge_index, edge_values, W1, b1, W2, b2, trace=False):
    p = Plan()
    meta = preprocess(p, edge_index, edge_values)
    nc = build_kernel(p, meta)

    features = np.asarray(features, np.float32)
    W1 = np.asarray(W1, np.float32)
    b1 = np.asarray(b1, np.float32).reshape(-1)
    W2 = np.asarray(W2, np.float32)
    b2 = np.asarray(b2, np.float32).reshape(-1)
    kchunks = p.f_in // 128
    sortpos = meta["sortpos"]

    in_maps = []
    for c in range(NCORES):
        fpad = np.zeros((p.rp, p.f_in), np.float32)
        fpad[sortpos[c]] = features[c * p.rpc:(c + 1) * p.rpc]
        ft = np.ascontiguousarray(fpad.T).reshape(kchunks, 128, p.rp)
        in_maps.append({
            "featT": np.ascontiguousarray(ft),
            "W1p": np.ascontiguousarray(W1.reshape(kchunks, 128, p.nhid)
                                        .transpose(1, 0, 2).reshape(128, kchunks * p.nhid)),
            "b1p": np.ascontiguousarray(b1.reshape(p.nhid, 1)),
            "W2p": np.ascontiguousarray(W2),
            "b2p": np.ascontiguousarray(np.broadcast_to(b2, (128, p.ncls))),
            "idxp": np.ascontiguousarray(meta["IDXW"][c]),
            "wp": np.ascontiguousarray(meta["WG"][c]),
        })

    res = run_bass_kernel_spmd(nc, in_maps, list(range(NCORES)), trace=trace)

    out = np.empty((p.n, p.ncls), np.float32)
    for c in range(NCORES):
        o = res.results[c]["out"]
        out[c * p.rpc:(c + 1) * p.rpc] = o[sortpos[c]]
    return out, res


def kernel(**inputs):
    out, _ = run(inputs["features"], inputs["edge_index"], inputs["edge_values"],
                 inputs["W1"], inputs["b1"], inputs["W2"], inputs["b2"])
    return out
